# revision 13
# baseline (speedup 1.0000x reference)
"""Trainium2 Bass kernel for the LSTM encoder/decoder problem.

Strategy (v2 — staggered decoupled chains):
- Pure data parallelism: 8 cores x 128 batch; per core two independent
  64-batch chains, software-staggered by half a step so every engine
  alternates between the chains and per-step latencies are hidden.
- Per chain-step: 4 fp16 matmuls (K=68: h rows + x/f + ones, bias folded),
  one per gate, all outputs at partitions 0:64 -> PSUM [64, 4*64] blocks
  [f|i|o|g]; ONE sigmoid activation covers all gates (g-rows pre-scaled
  by 2 so tanh(g) = 2*sig(2g)-1). Same-partition-base layout keeps every
  elementwise op all-SBUF all-fp16 (DVE 2x mode, no PSUM operands).
- Cell state kept as C = c/2 (fp16; forget gates keep |c| bounded):
    w  = (S_g - 0.5) * S_i           (DVE STT)
    v  = S_f * C                     (DVE TT)
    C' = v + w                       (DVE TT)
    Y  = sigmoid(4*C')               (scalar ACT, = sig(2c'))
    H  = (Y - 0.5) * S_o             (DVE STT) == h/2
  The tanh+h-mult pair of the classic formulation collapses into one
  sigmoid + one STT; the 2x is folded into the h-columns of the
  recurrence weights and into W_out.
- Decoder output projection folded into recurrence (W_eff) as before;
  y computed by [66,512] matmuls inlined every 4th decoder step, DMA'd
  straight from PSUM to DRAM on the sync engine.
- Encoder x rows stream into a 16-slot rhs ring via sync-engine DMAs
  every 8 steps. All 2-byte tensors are fp16 (precision + DVE 2x mode).
"""
import sys

import numpy as np

sys.path.insert(0, "/opt/trn_rl_repo")

from concourse import bacc, mybir, tile  # noqa: E402
from concourse.bass_utils import run_bass_kernel_spmd  # noqa: E402

H = 64
IN = 3
OUT = 2
B = 1024
T = 512
PL = 300
NCORES = 8
BS = B // NCORES          # 128 batch per core
CB = BS // 2              # 64 batch per chain
NSTEPS = T + PL - 1       # 811
NSLOTS = 16               # encoder rhs ring slots
HCOLS = PL * BS           # 38400 history columns
F32 = mybir.dt.float32
F16 = mybir.dt.float16
ALU = mybir.AluOpType
ACT = mybir.ActivationFunctionType

_cache = {}
_last_in_maps = None


def _build_program():
    nc = bacc.Bacc(None)
    xrows = nc.declare_dram_parameter("xrows", [4, T, BS], F16, isOutput=False)
    histi = nc.declare_dram_parameter("histi", [2, HCOLS], F16, isOutput=False)
    winit = nc.declare_dram_parameter("winit", [68, 514], F16, isOutput=False)
    yout = nc.declare_dram_parameter("y", [BS, PL * OUT], F32, isOutput=True)

    with tile.TileContext(nc) as tc:
        with (
            tc.tile_pool(name="pool", bufs=1) as pool,
            tc.tile_pool(name="psum", bufs=1, space="PSUM") as pp,
        ):
            mega = pool.tile([68, NSLOTS, BS], F16, tag="mega")
            hist = pool.tile([66, HCOLS], F16, tag="hist")
            winit_t = pool.tile([68, 514], F16, tag="winit")
            S = [[pool.tile([64, 4, CB], F16, tag=f"S{c}{i}", name=f"S{c}{i}")
                  for i in range(6)] for c in range(2)]
            C = [[pool.tile([64, CB], F16, tag=f"C{c}{i}", name=f"C{c}{i}")
                  for i in range(2)] for c in range(2)]
            Y = [[pool.tile([64, CB], F16, tag=f"Y{c}{i}", name=f"Y{c}{i}")
                  for i in range(2)] for c in range(2)]
            vt = [[pool.tile([64, CB], F16, tag=f"v{c}{i}", name=f"vt{c}{i}")
                   for i in range(2)] for c in range(2)]
            wt = [[pool.tile([64, CB], F16, tag=f"wt{c}{i}", name=f"wt{c}{i}")
                   for i in range(2)] for c in range(2)]
            P = [[pp.tile([64, 4, CB], F32, tag=f"P{c}{i}", name=f"P{c}{i}")
                  for i in range(2)] for c in range(2)]
            yps = pp.tile([BS, 2, OUT], F32, tag="yps", name="yps")
            ysb = pool.tile([BS, PL * OUT], F32, tag="ysb")

            # ---- init ----
            nc.sync.dma_start(winit_t[:], winit[:])
            nc.sync.dma_start(hist[64:66, :], histi[:])
            nc.gpsimd.dma_start(mega[64:68, 0:NSLOTS, :], xrows[:, 0:NSLOTS, :])
            nc.gpsimd.memset(mega[0:64, 0, :], 0.0)  # H0 = 0
            for c in range(2):
                nc.gpsimd.memset(C[c][0][:], 0.0)    # c0 = 0

            def h1(ch, t):
                """MMs + gate sigmoid + w + v for chain ch, step t."""
                par = t % 2
                cs = slice(ch * CB, (ch + 1) * CB)
                if t < T:
                    rhs = mega[:, t % NSLOTS, cs]
                    wcol = 0
                else:
                    d = t - T
                    rhs = hist[:, d * BS + ch * CB:d * BS + (ch + 1) * CB]
                    wcol = 256
                Pc = P[ch][par]
                Sc = S[ch][t % 6]
                nk = rhs.partition_size()
                for g in range(4):
                    nc.tensor.matmul(
                        Pc[:, g, :], winit_t[0:nk, wcol + 64 * g:wcol + 64 * (g + 1)],
                        rhs, start=True, stop=True,
                    )
                # split sigmoid: {g,i,f} feeds w/v right away; {o} is only
                # needed by the h STT after the cell sigmoid -> off-path
                nc.scalar.activation(Sc[:, 0:3, :], Pc[:, 0:3, :], ACT.Sigmoid)
                nc.scalar.activation(Sc[:, 3, :], Pc[:, 3, :], ACT.Sigmoid)
                # w = (S_g - 0.5) * S_i
                nc.vector.scalar_tensor_tensor(
                    wt[ch][par][:], Sc[:, 0, :], 0.5, Sc[:, 1, :],
                    ALU.subtract, ALU.mult,
                )
                # v = S_f * C
                nc.vector.tensor_tensor(
                    vt[ch][par][:], Sc[:, 2, :], C[ch][par][:], ALU.mult
                )

            def h2(ch, t):
                """C' add + cell sigmoid + H for chain ch, step t."""
                par = t % 2
                nxt = (t + 1) % 2
                cs = slice(ch * CB, (ch + 1) * CB)
                Sc = S[ch][t % 6]
                # C' = v + w
                nc.vector.tensor_tensor(
                    C[ch][nxt][:], vt[ch][par][:], wt[ch][par][:], ALU.add
                )
                # Y = sigmoid(4*C') == sig(2c')
                nc.scalar.activation(
                    Y[ch][par][:], C[ch][nxt][:], ACT.Sigmoid, scale=4.0
                )
                # H = (Y - 0.5) * S_o == h/2
                if t < T - 1:
                    htgt = mega[0:64, (t + 1) % NSLOTS, cs]
                else:
                    d = t - (T - 1)
                    htgt = hist[0:64, d * BS + ch * CB:d * BS + (ch + 1) * CB]
                nc.vector.scalar_tensor_tensor(
                    htgt, Y[ch][par][:], 0.5, Sc[:, 3, :],
                    ALU.subtract, ALU.mult,
                )

            def ychunk(d):
                # y_d^T: stationary = hist block d [66, 128], moving = wy
                q = d % 2
                cols = slice(d * BS, (d + 1) * BS)
                nc.tensor.matmul(
                    yps[:, q, :], hist[:, cols], winit_t[0:66, 512:514],
                    start=True, stop=True
                )
                nc.vector.tensor_copy(
                    ysb[:, OUT * d:OUT * (d + 1)], yps[:, q, :]
                )

            # ---- recurrence: chain 0 leads, chain 1 staggered half a step
            for t in range(NSTEPS):
                if t < T and t % 8 == 0 and 8 <= t + 8 < T:
                    t0 = t + 8
                    s0 = t0 % NSLOTS
                    nc.sync.dma_start(
                        mega[64:68, s0:s0 + 8, :], xrows[:, t0:t0 + 8, :]
                    )
                if t > 0:
                    h2(1, t - 1)
                    # hist block d complete after h2(1, T-1+d)
                    if t >= T:
                        ychunk(t - T)
                h1(0, t)
                h2(0, t)
                h1(1, t)
            h2(1, NSTEPS - 1)
            ychunk(PL - 1)
            nc.sync.dma_start(yout[:], ysb[:])
    nc.finalize()
    return nc


def _prep_weights(W_ih, W_hh, b_ih, b_hh, W_out, b_out):
    b = (b_ih + b_hh).astype(np.float32)
    W_eff = (W_ih[:, :2] @ W_out + W_hh).astype(np.float32)
    b_eff = (b + W_ih[:, :2] @ b_out).astype(np.float32)
    w_f = W_ih[:, 2].astype(np.float32)
    # pytorch gate order i,f,g,o -> column blocks [g, i, f, o]
    perm = np.concatenate([np.arange(128, 192), np.arange(0, 64),
                           np.arange(64, 128), np.arange(192, 256)])
    scale = np.ones(256, np.float32)
    scale[0:64] = 2.0  # g rows: sigmoid(2g) trick
    # h columns x2: rhs carries H = h/2
    wenc = np.ascontiguousarray(
        (np.concatenate([2.0 * W_hh, W_ih, b[:, None]], 1)[perm]
         * scale[:, None]).T
    ).astype(np.float32)
    wdec = np.ascontiguousarray(
        (np.concatenate([2.0 * W_eff, w_f[:, None], b_eff[:, None]], 1)[perm]
         * scale[:, None]).T
    ).astype(np.float32)
    wy = np.concatenate(
        [2.0 * W_out.T, np.zeros((1, OUT), np.float32), b_out[None, :]], 0
    ).astype(np.float32)
    winit = np.zeros((68, 514), np.float32)
    winit[:, 0:256] = wenc
    winit[0:66, 256:512] = wdec
    winit[0:66, 512:514] = wy
    return winit.astype(np.float16)


def kernel(x, force, W_ih, W_hh, b_ih, b_hh, W_out, b_out, predict_length):
    assert int(predict_length) == PL
    x = np.asarray(x, np.float32)
    force = np.asarray(force, np.float32)
    winit = _prep_weights(
        np.asarray(W_ih, np.float32), np.asarray(W_hh, np.float32),
        np.asarray(b_ih, np.float32), np.asarray(b_hh, np.float32),
        np.asarray(W_out, np.float32), np.asarray(b_out, np.float32),
    )

    if "nc" not in _cache:
        _cache["nc"] = _build_program()
    nc = _cache["nc"]

    in_maps = []
    for c in range(NCORES):
        sl = slice(c * BS, (c + 1) * BS)
        xs = x[sl]                                  # [BS, T, 3]
        xrows = np.ones((4, T, BS), np.float32)
        xrows[0:3] = xs.transpose(2, 1, 0)
        fs = force[sl, :, 0]                        # [BS, 299]
        histi = np.zeros((2, HCOLS), np.float32)
        histi[0, :(PL - 1) * BS] = fs.T.ravel()
        histi[1] = 1.0
        in_maps.append({
            "xrows": xrows.astype(np.float16),
            "histi": histi.astype(np.float16),
            "winit": winit,
        })

    global _last_in_maps
    _last_in_maps = in_maps
    res = run_bass_kernel_spmd(nc, in_maps, list(range(NCORES)))
    outs = []
    for c in range(NCORES):
        yc = res.results[c]["y"]                    # [BS, PL*OUT]
        outs.append(yc.reshape(BS, PL, OUT))
    return np.ascontiguousarray(np.concatenate(outs, 0)).astype(np.float32)


# revision 14
# speedup vs baseline: 1.0014x; 1.0014x over previous
"""Trainium2 Bass kernel for the LSTM encoder/decoder problem.

Strategy (v2 — staggered decoupled chains):
- Pure data parallelism: 8 cores x 128 batch; per core two independent
  64-batch chains, software-staggered by half a step so every engine
  alternates between the chains and per-step latencies are hidden.
- Per chain-step: 4 fp16 matmuls (K=68: h rows + x/f + ones, bias folded),
  one per gate, all outputs at partitions 0:64 -> PSUM [64, 4*64] blocks
  [f|i|o|g]; ONE sigmoid activation covers all gates (g-rows pre-scaled
  by 2 so tanh(g) = 2*sig(2g)-1). Same-partition-base layout keeps every
  elementwise op all-SBUF all-fp16 (DVE 2x mode, no PSUM operands).
- Cell state kept as C = c/2 (fp16; forget gates keep |c| bounded):
    w  = (S_g - 0.5) * S_i           (DVE STT)
    v  = S_f * C                     (DVE TT)
    C' = v + w                       (DVE TT)
    Y  = sigmoid(4*C')               (scalar ACT, = sig(2c'))
    H  = (Y - 0.5) * S_o             (DVE STT) == h/2
  The tanh+h-mult pair of the classic formulation collapses into one
  sigmoid + one STT; the 2x is folded into the h-columns of the
  recurrence weights and into W_out.
- Decoder output projection folded into recurrence (W_eff) as before;
  y computed by [66,512] matmuls inlined every 4th decoder step, DMA'd
  straight from PSUM to DRAM on the sync engine.
- Encoder x rows stream into a 16-slot rhs ring via sync-engine DMAs
  every 8 steps. All 2-byte tensors are fp16 (precision + DVE 2x mode).
"""
import sys

import numpy as np

sys.path.insert(0, "/opt/trn_rl_repo")

from concourse import bacc, mybir, tile  # noqa: E402
from concourse.bass_utils import run_bass_kernel_spmd  # noqa: E402

H = 64
IN = 3
OUT = 2
B = 1024
T = 512
PL = 300
NCORES = 8
BS = B // NCORES          # 128 batch per core
CB = BS // 2              # 64 batch per chain
NSTEPS = T + PL - 1       # 811
NSLOTS = 16               # encoder rhs ring slots
HCOLS = PL * BS           # 38400 history columns
F32 = mybir.dt.float32
F16 = mybir.dt.float16
ALU = mybir.AluOpType
ACT = mybir.ActivationFunctionType

_cache = {}
_last_in_maps = None


def _build_program():
    nc = bacc.Bacc(None)
    xrows = nc.declare_dram_parameter("xrows", [4, T, BS], F16, isOutput=False)
    histi = nc.declare_dram_parameter("histi", [2, HCOLS], F16, isOutput=False)
    winit = nc.declare_dram_parameter("winit", [68, 514], F16, isOutput=False)
    yout = nc.declare_dram_parameter("y", [BS, PL * OUT], F32, isOutput=True)

    with tile.TileContext(nc) as tc:
        with (
            tc.tile_pool(name="pool", bufs=1) as pool,
            tc.tile_pool(name="psum", bufs=1, space="PSUM") as pp,
        ):
            mega = pool.tile([68, NSLOTS, BS], F16, tag="mega")
            hist = pool.tile([66, HCOLS], F16, tag="hist")
            winit_t = pool.tile([68, 514], F16, tag="winit")
            S = [[pool.tile([64, 4, CB], F16, tag=f"S{c}{i}", name=f"S{c}{i}")
                  for i in range(6)] for c in range(2)]
            C = [[pool.tile([64, CB], F16, tag=f"C{c}{i}", name=f"C{c}{i}")
                  for i in range(2)] for c in range(2)]
            Y = [[pool.tile([64, CB], F16, tag=f"Y{c}{i}", name=f"Y{c}{i}")
                  for i in range(2)] for c in range(2)]
            vt = [[pool.tile([64, CB], F16, tag=f"v{c}{i}", name=f"vt{c}{i}")
                   for i in range(2)] for c in range(2)]
            wt = [[pool.tile([64, CB], F16, tag=f"wt{c}{i}", name=f"wt{c}{i}")
                   for i in range(2)] for c in range(2)]
            P = [[pp.tile([64, 4, CB], F32, tag=f"P{c}{i}", name=f"P{c}{i}")
                  for i in range(2)] for c in range(2)]
            yps = [pp.tile([BS, PL], F32, tag=f"yps{i}", name=f"yps{i}")
                   for i in range(2)]
            ysb = pool.tile([BS, PL * OUT], F32, tag="ysb")

            # ---- init ----
            nc.sync.dma_start(winit_t[:], winit[:])
            nc.sync.dma_start(hist[64:66, :], histi[:])
            nc.gpsimd.dma_start(mega[64:68, 0:NSLOTS, :], xrows[:, 0:NSLOTS, :])
            nc.gpsimd.memset(mega[0:64, 0, :], 0.0)  # H0 = 0
            for c in range(2):
                nc.gpsimd.memset(C[c][0][:], 0.0)    # c0 = 0

            def h1(ch, t):
                """MMs + gate sigmoid + w + v for chain ch, step t."""
                par = t % 2
                cs = slice(ch * CB, (ch + 1) * CB)
                if t < T:
                    rhs = mega[:, t % NSLOTS, cs]
                    wcol = 0
                else:
                    d = t - T
                    rhs = hist[:, d * BS + ch * CB:d * BS + (ch + 1) * CB]
                    wcol = 256
                Pc = P[ch][par]
                Sc = S[ch][t % 6]
                nk = rhs.partition_size()
                for g in range(4):
                    nc.tensor.matmul(
                        Pc[:, g, :], winit_t[0:nk, wcol + 64 * g:wcol + 64 * (g + 1)],
                        rhs, start=True, stop=True,
                    )
                # split sigmoid: {g,i,f} feeds w/v right away; {o} is only
                # needed by the h STT after the cell sigmoid -> off-path
                nc.scalar.activation(Sc[:, 0:3, :], Pc[:, 0:3, :], ACT.Sigmoid)
                nc.scalar.activation(Sc[:, 3, :], Pc[:, 3, :], ACT.Sigmoid)
                # w = (S_g - 0.5) * S_i
                nc.vector.scalar_tensor_tensor(
                    wt[ch][par][:], Sc[:, 0, :], 0.5, Sc[:, 1, :],
                    ALU.subtract, ALU.mult,
                )
                # v = S_f * C
                nc.vector.tensor_tensor(
                    vt[ch][par][:], Sc[:, 2, :], C[ch][par][:], ALU.mult
                )

            def h2(ch, t):
                """C' add + cell sigmoid + H for chain ch, step t."""
                par = t % 2
                nxt = (t + 1) % 2
                cs = slice(ch * CB, (ch + 1) * CB)
                Sc = S[ch][t % 6]
                # C' = v + w
                nc.vector.tensor_tensor(
                    C[ch][nxt][:], vt[ch][par][:], wt[ch][par][:], ALU.add
                )
                # Y = sigmoid(4*C') == sig(2c')
                nc.scalar.activation(
                    Y[ch][par][:], C[ch][nxt][:], ACT.Sigmoid, scale=4.0
                )
                # H = (Y - 0.5) * S_o == h/2
                if t < T - 1:
                    htgt = mega[0:64, (t + 1) % NSLOTS, cs]
                else:
                    d = t - (T - 1)
                    htgt = hist[0:64, d * BS + ch * CB:d * BS + (ch + 1) * CB]
                nc.vector.scalar_tensor_tensor(
                    htgt, Y[ch][par][:], 0.5, Sc[:, 3, :],
                    ALU.subtract, ALU.mult,
                )

            def ychunk(d):
                # y_d^T: stationary = hist block d [66, 128], moving = wy;
                # results accumulate across steps in two PSUM banks, copied
                # out once after the loop (no per-step engine copy).
                q, r = divmod(d, PL // 2)
                cols = slice(d * BS, (d + 1) * BS)
                nc.tensor.matmul(
                    yps[q][:, OUT * r:OUT * (r + 1)], hist[:, cols],
                    winit_t[0:66, 512:514], start=True, stop=True
                )

            # ---- recurrence: chain 0 leads, chain 1 staggered half a step
            for t in range(NSTEPS):
                if t < T and t % 8 == 0 and 8 <= t + 8 < T:
                    t0 = t + 8
                    s0 = t0 % NSLOTS
                    nc.sync.dma_start(
                        mega[64:68, s0:s0 + 8, :], xrows[:, t0:t0 + 8, :]
                    )
                if t > 0:
                    h2(1, t - 1)
                    # hist block d complete after h2(1, T-1+d)
                    if t >= T:
                        ychunk(t - T)
                h1(0, t)
                h2(0, t)
                h1(1, t)
            h2(1, NSTEPS - 1)
            ychunk(PL - 1)
            for q in range(2):
                nc.vector.tensor_copy(
                    ysb[:, q * PL:(q + 1) * PL], yps[q][:]
                )
            nc.sync.dma_start(yout[:], ysb[:])
    nc.finalize()
    return nc


def _prep_weights(W_ih, W_hh, b_ih, b_hh, W_out, b_out):
    b = (b_ih + b_hh).astype(np.float32)
    W_eff = (W_ih[:, :2] @ W_out + W_hh).astype(np.float32)
    b_eff = (b + W_ih[:, :2] @ b_out).astype(np.float32)
    w_f = W_ih[:, 2].astype(np.float32)
    # pytorch gate order i,f,g,o -> column blocks [g, i, f, o]
    perm = np.concatenate([np.arange(128, 192), np.arange(0, 64),
                           np.arange(64, 128), np.arange(192, 256)])
    scale = np.ones(256, np.float32)
    scale[0:64] = 2.0  # g rows: sigmoid(2g) trick
    # h columns x2: rhs carries H = h/2
    wenc = np.ascontiguousarray(
        (np.concatenate([2.0 * W_hh, W_ih, b[:, None]], 1)[perm]
         * scale[:, None]).T
    ).astype(np.float32)
    wdec = np.ascontiguousarray(
        (np.concatenate([2.0 * W_eff, w_f[:, None], b_eff[:, None]], 1)[perm]
         * scale[:, None]).T
    ).astype(np.float32)
    wy = np.concatenate(
        [2.0 * W_out.T, np.zeros((1, OUT), np.float32), b_out[None, :]], 0
    ).astype(np.float32)
    winit = np.zeros((68, 514), np.float32)
    winit[:, 0:256] = wenc
    winit[0:66, 256:512] = wdec
    winit[0:66, 512:514] = wy
    return winit.astype(np.float16)


def kernel(x, force, W_ih, W_hh, b_ih, b_hh, W_out, b_out, predict_length):
    assert int(predict_length) == PL
    x = np.asarray(x, np.float32)
    force = np.asarray(force, np.float32)
    winit = _prep_weights(
        np.asarray(W_ih, np.float32), np.asarray(W_hh, np.float32),
        np.asarray(b_ih, np.float32), np.asarray(b_hh, np.float32),
        np.asarray(W_out, np.float32), np.asarray(b_out, np.float32),
    )

    if "nc" not in _cache:
        _cache["nc"] = _build_program()
    nc = _cache["nc"]

    in_maps = []
    for c in range(NCORES):
        sl = slice(c * BS, (c + 1) * BS)
        xs = x[sl]                                  # [BS, T, 3]
        xrows = np.ones((4, T, BS), np.float32)
        xrows[0:3] = xs.transpose(2, 1, 0)
        fs = force[sl, :, 0]                        # [BS, 299]
        histi = np.zeros((2, HCOLS), np.float32)
        histi[0, :(PL - 1) * BS] = fs.T.ravel()
        histi[1] = 1.0
        in_maps.append({
            "xrows": xrows.astype(np.float16),
            "histi": histi.astype(np.float16),
            "winit": winit,
        })

    global _last_in_maps
    _last_in_maps = in_maps
    res = run_bass_kernel_spmd(nc, in_maps, list(range(NCORES)))
    outs = []
    for c in range(NCORES):
        yc = res.results[c]["y"]                    # [BS, 2*150*OUT]
        outs.append(yc.reshape(BS, PL, OUT))
    return np.ascontiguousarray(np.concatenate(outs, 0)).astype(np.float32)
